# revision 85
# baseline (speedup 1.0000x reference)
"""Luong attention kernel for Trainium2, data-parallel over 8 NeuronCores.

Problem (per full input):
    hidden          [1, 64, 2048] f32   -> q = hidden[0]           [B, H]
    encoder_outputs [64, 2048, 2048] f32                           [B, S, H]
    scores[b, s] = <enc[b, s, :], q[b, :]>
    attn = softmax(scores, axis=1)                                 [B, S]
    context[b, h] = sum_s attn[b, s] * enc[b, s, h]                [B, H]
    returns (context, attn)

Sharding: pure data parallel on B: each of the 8 cores handles 8 batches.

Per-core kernel design (memory-bound; E = enc slice is read from HBM exactly
once, staying resident in SBUF for both the scores pass and the context pass;
modeled per-core time 296.3us, DVE-bound):
  for each local batch b (8 per core):
    - q broadcast to 128 partitions via ones-matmul into PSUM + copy to SBUF
    - E[b] loaded in 8 chunks of [128, 2, 2048] (2 MiB DMAs) alternating
      between the SP HWDGE and Pool SWDGE queues — the two DGE queues
      transfer concurrently, roughly halving the DMA wall; the last
      batch switches to per-tile DMAs into slices of the same slots
      (subtile deps) with a geometric taper of the final two tiles, so the
      DVE score train is arrival-paced and the post-last-byte tail is just
      sem + one small piece
    - scores per s-tile via one fused DVE pass: scalar_tensor_tensor
      (bypass/mult) with accum_out row-sum (single pass over E; the
      InstTensorTensorReduce / InstPartitionAllReduce ISA ops are NOT
      supported by this walrus build)
    - softmax: global max via GPSIMD cross-partition reduce + negating
      ones-matmul broadcast; exp on ACT WITHOUT accum (the ACT accumulator
      read costs 187ns); row-sum on DVE; global sum + broadcast fused into
      a single [P,P]-ones matmul; reciprocal on DVE
    - context via PE matmuls accumulating in PSUM [128, 16] (h-major
      layout); the attn/ctx stores go on different DGE queues; the final
      batch's ctx store uses the (by then idle) SP queue for its lower
      completion-sem delay
    - outputs stored in [128, 16] partition layout; host reassembles
  Buffering: epool bufs=10 (2-chunk prefetch slack — 9 bufs costs +41us;
  buffer recycling gates the dual DMA queues). Two score tiles per batch
  ({1,9}, from SP-loaded chunks) run on Pool (tensor_tensor mult into an
  SBUF scratch) + ACT (in-place Copy activation with accum_out row-sum)
  instead of DVE, shaving the DVE bottleneck; the scratch is funded by
  qpool=1 (the train-start delay is absorbed in the DVE-bound regime).
  NOTE: the PSUM variant of that scratch fails walrus BIR verification.
"""

import numpy as np

B, S, H = 64, 2048, 2048
NCORES = 8
BL = B // NCORES          # local batches per core
P = 128                   # SBUF partitions
NT = S // P               # 16 s-tiles per batch
UCH = 2                   # s-tiles per DMA chunk
NCHUNK = NT // UCH        # 8 chunks per batch
HC = H // P               # 16 h-chunks for the context matmul
NQ = H // 512             # q broadcast chunks

_NC = None


def _build_nc():
    import concourse.bass as bass
    import concourse.tile as tile
    from concourse import mybir

    F32 = mybir.dt.float32
    Alu = mybir.AluOpType
    Act = mybir.ActivationFunctionType

    nc = bass.Bass()
    hid = nc.declare_dram_parameter("hidden", [BL, H], F32, isOutput=False)
    enc = nc.declare_dram_parameter("enc", [BL, S, H], F32, isOutput=False)
    ctx_out = nc.declare_dram_parameter("ctx_out", [BL, P, HC], F32, isOutput=True)
    attn_out = nc.declare_dram_parameter("attn_out", [BL, P, NT], F32, isOutput=True)

    with tile.TileContext(nc) as tc:
        with (
            tc.tile_pool(name="consts", bufs=1) as consts,
            tc.tile_pool(name="epool", bufs=10) as epool,
            tc.tile_pool(name="qpool", bufs=1) as qpool,
            tc.tile_pool(name="qrowp", bufs=1) as qrowp,
            tc.tile_pool(name="tmpp", bufs=1) as tmpp,
            tc.tile_pool(name="ptmp", bufs=1) as ptmp,
            tc.tile_pool(name="smallp", bufs=3) as smallp,
            tc.tile_pool(name="psqp", bufs=2, space="PSUM") as psqp,
            tc.tile_pool(name="psbp", bufs=2, space="PSUM") as psbp,
            tc.tile_pool(name="pscp", bufs=2, space="PSUM") as pscp,
        ):
            ones_row = consts.tile([1, P], F32, tag="ones")
            nc.vector.memset(ones_row, 1.0)
            neg_ones_row = consts.tile([1, P], F32, tag="negones")
            nc.vector.memset(neg_ones_row, -1.0)
            ones_mat = consts.tile([P, P], F32, tag="onesmat")
            nc.vector.memset(ones_mat, 1.0)
            for b in range(BL):
                # --- q[b] -> broadcast across all 128 partitions via PE ---
                q_row = qrowp.tile([1, H], F32, tag="qrow")
                nc.gpsimd.dma_start(out=q_row, in_=hid[b : b + 1, :])
                q_sb = qpool.tile([P, H], F32, tag="qsb")
                for j in range(NQ):
                    psq = psqp.tile([P, 512], F32, tag="psq")
                    nc.tensor.matmul(
                        psq,
                        lhsT=ones_row,
                        rhs=q_row[:, j * 512 : (j + 1) * 512],
                        start=True,
                        stop=True,
                    )
                    nc.scalar.copy(out=q_sb[:, j * 512 : (j + 1) * 512], in_=psq)

                # --- stream E[b]; fused dot-products for scores ---
                # Last batch: geometric taper of the final two tiles. The
                # tail after the last byte is sem(~1.05us) + remaining DVE
                # work; smaller final pieces minimize the max over pieces of
                # (arrival + remaining work), balancing DMA rate (1.44x the
                # DVE rate) against per-piece overhead.
                scores = smallp.tile([P, NT], F32, tag="scores")
                last = b == BL - 1

                def dot(dst_eh, t, h0, h1, accum):
                    # DMA enc[b, tile t, h0:h1] into dst_eh and run the
                    # fused mul+row-sum pass into accum.
                    nc.sync.dma_start(
                        out=dst_eh, in_=enc[b, t * P : (t + 1) * P, h0:h1]
                    )
                    tmp = tmpp.tile([P, H], F32, tag="tmp")
                    nc.vector.scalar_tensor_tensor(
                        out=tmp[:, 0 : h1 - h0],
                        in0=dst_eh,
                        scalar=1.0,
                        in1=q_sb[:, h0:h1],
                        op0=Alu.bypass,
                        op1=Alu.mult,
                        accum_out=accum,
                    )

                # tiles whose dot runs on Pool+ACT instead of DVE (from
                # SP-loaded chunks so Pool's DMA queue never self-blocks)
                offload = {1, 9} if not last else set()
                e_chunks = []   # (ech, tbase, w) covering full-H tiles
                if not last:
                    for u in range(NCHUNK):
                        t0 = u * UCH
                        ech = epool.tile([P, UCH, H], F32, tag="E")
                        src = enc[b, t0 * P : (t0 + UCH) * P, :].rearrange(
                            "(a p) h -> p a h", p=P
                        )
                        eng = nc.sync if u % 2 == 0 else nc.gpsimd
                        eng.dma_start(out=ech, in_=src)
                        e_chunks.append((ech, t0, UCH))
                        for k in range(UCH):
                            t = t0 + k
                            if t in offload:
                                # DVE is the bottleneck: run this tile's dot
                                # as Pool-mul (PSUM scratch) + ACT row-sum
                                pm = ptmp.tile([P, H], F32, tag="pm")
                                nc.gpsimd.tensor_tensor(
                                    out=pm, in0=ech[:, k, :], in1=q_sb,
                                    op=Alu.mult,
                                )
                                nc.scalar.activation(
                                    out=pm, in_=pm, func=Act.Copy, scale=1.0,
                                    accum_out=scores[:, t : t + 1],
                                )
                                continue
                            tmp = tmpp.tile([P, H], F32, tag="tmp")
                            nc.vector.scalar_tensor_tensor(
                                out=tmp,
                                in0=ech[:, k, :],
                                scalar=1.0,
                                in1=q_sb,
                                op0=Alu.bypass,
                                op1=Alu.mult,
                                accum_out=scores[:, t : t + 1],
                            )
                else:
                    # Last batch: per-tile (and, for the final two tiles,
                    # sub-tile) DMA granularity so the DVE score train is
                    # arrival-paced and the post-last-byte tail is just
                    # sem + one small piece. Same 8 [P,2,H] slots: DMAs
                    # target disjoint slices (subtile deps).
                    for u in range(NCHUNK - 1):
                        ech = epool.tile([P, UCH, H], F32, tag="E")
                        e_chunks.append((ech, u * UCH, UCH))
                        for k in range(UCH):
                            t = u * UCH + k
                            dot(ech[:, k, :], t, 0, H, scores[:, t : t + 1])
                    slot7 = epool.tile([P, UCH, H], F32, tag="E")
                    t14, t15 = NT - 2, NT - 1
                    sh14 = smallp.tile([P, 2], F32, tag="sh14")
                    for i, (h0, h1) in enumerate([(0, 1024), (1024, 2048)]):
                        dot(slot7[:, 0, h0:h1], t14, h0, h1, sh14[:, i : i + 1])
                    nc.vector.tensor_add(
                        scores[:, t14 : t14 + 1], sh14[:, 0:1], sh14[:, 1:2]
                    )
                    segs15 = [(0, 896), (896, 1408), (1408, 1792), (1792, 2048)]
                    sh15 = smallp.tile([P, len(segs15)], F32, tag="sh15")
                    for i, (h0, h1) in enumerate(segs15):
                        dot(slot7[:, 1, h0:h1], t15, h0, h1, sh15[:, i : i + 1])
                    nc.vector.tensor_reduce(
                        out=scores[:, t15 : t15 + 1],
                        in_=sh15,
                        axis=mybir.AxisListType.X,
                        op=Alu.add,
                    )
                    e_chunks.append((slot7, NT - 2, UCH))
                # --- softmax over the 2048 scores ---
                m1 = smallp.tile([P, 1], F32, tag="m1")
                nc.vector.tensor_reduce(
                    out=m1, in_=scores, axis=mybir.AxisListType.X, op=Alu.max
                )
                gmax = smallp.tile([1, 1], F32, tag="gmax")
                nc.gpsimd.tensor_reduce(
                    out=gmax, in_=m1, axis=mybir.AxisListType.XYZWC, op=Alu.max
                )
                psb1 = psbp.tile([P, 1], F32, tag="psb")
                nc.tensor.matmul(psb1, lhsT=neg_ones_row, rhs=gmax, start=True, stop=True)
                negmax_sb = smallp.tile([P, 1], F32, tag="negmax")
                nc.scalar.copy(out=negmax_sb, in_=psb1)

                e_col = smallp.tile([P, NT], F32, tag="ecol")
                row_sums = smallp.tile([P, 1], F32, tag="rowsums")
                # no accum_out on the exp (ACT accumulator read costs 187ns
                # on the critical path); row_sums on DVE runs in parallel
                # with the PE context train
                nc.scalar.activation(
                    out=e_col, in_=scores, func=Act.Exp, bias=negmax_sb, scale=1.0
                )
                nc.vector.tensor_reduce(
                    out=row_sums, in_=e_col, axis=mybir.AxisListType.X, op=Alu.add
                )
                # sum+broadcast of row_sums in one matmul: [P,P] ones lhsT
                psb2 = psbp.tile([P, 1], F32, tag="psb")
                nc.tensor.matmul(psb2, lhsT=ones_mat, rhs=row_sums, start=True, stop=True)
                rec_sb = smallp.tile([P, 1], F32, tag="recsb")
                nc.vector.reciprocal(rec_sb, psb2)

                attn_sb = smallp.tile([P, NT], F32, tag="attnsb")
                nc.vector.tensor_scalar_mul(out=attn_sb, in0=e_col, scalar1=rec_sb)
                nc.gpsimd.dma_start(out=attn_out[b], in_=attn_sb)

                # --- context: accumulate over all s-tiles in PSUM [128, 16] ---
                # Column-outer order: each h-column's accumulation group
                # completes independently, so the first half can be scaled
                # and stored while the PE still works on the second half.
                psc = pscp.tile([P, HC], F32, tag="psc")
                lhs_of = {}
                for ech, tbase, w in e_chunks:
                    for k in range(w):
                        lhs_of[tbase + k] = (ech, k)
                ctx_sb = smallp.tile([P, HC], F32, tag="ctxsb")
                for c in range(HC):
                    for t in range(NT):
                        ech, k = lhs_of[t]
                        nc.tensor.matmul(
                            psc[:, c : c + 1],
                            lhsT=ech[:, k, c * P : (c + 1) * P],
                            rhs=e_col[:, t : t + 1],
                            start=(t == 0),
                            stop=(t == NT - 1),
                        )
                nc.vector.tensor_scalar_mul(out=ctx_sb, in0=psc, scalar1=rec_sb)
                # different DGE engine than attn_out so the stores overlap.
                # Final batch: SP's queue has drained all E-loads by now and
                # has the lowest completion-sem delay (650 vs 784).
                if last:
                    nc.sync.dma_start(out=ctx_out[b], in_=ctx_sb)
                else:
                    nc.scalar.dma_start(out=ctx_out[b], in_=ctx_sb)

    _split_waits(nc)
    return nc


def _split_waits(nc, maxw=1):
    """This walrus build accepts at most one semaphore wait per instruction;
    move extra waits onto NoOp carriers inserted just before (same engine)."""
    from concourse import mybir

    nsplit = 0
    for bb in nc.main_func.blocks:
        newlist = []
        for ins in bb.instructions:
            si = ins.sync_info
            if si is not None and si.on_wait and len(si.on_wait) > maxw:
                waits = list(si.on_wait)
                chunks = [waits[i : i + maxw] for i in range(0, len(waits), maxw)]
                for chunk in chunks[:-1]:
                    pre = mybir.InstNoOp(
                        name=f"{ins.name}_wsplit{nsplit}",
                        engine=ins.engine,
                        ins=[],
                        outs=[],
                        sync_info=mybir.SyncInfo(on_wait=chunk, on_update=[]),
                    )
                    nsplit += 1
                    nc.register_instruction(pre, overwrite=True)
                    newlist.append(pre)
                ins.sync_info = mybir.SyncInfo(
                    on_wait=chunks[-1], on_update=list(si.on_update or [])
                )
            newlist.append(ins)
        bb.instructions[:] = newlist
    return nsplit


def get_nc():
    global _NC
    if _NC is None:
        _NC = _build_nc()
    return _NC


def make_in_maps(hidden, encoder_outputs):
    q = np.asarray(hidden, dtype=np.float32).reshape(B, H)
    enc = np.asarray(encoder_outputs, dtype=np.float32)
    in_maps = []
    for i in range(NCORES):
        in_maps.append(
            {
                "hidden": np.ascontiguousarray(q[i * BL : (i + 1) * BL]),
                "enc": np.ascontiguousarray(enc[i * BL : (i + 1) * BL]),
            }
        )
    return in_maps


def assemble(results):
    ctx = np.concatenate(
        [r["ctx_out"].transpose(0, 2, 1).reshape(BL, H) for r in results]
    )
    attn = np.concatenate(
        [r["attn_out"].transpose(0, 2, 1).reshape(BL, S) for r in results]
    )
    return ctx.astype(np.float32), attn.astype(np.float32)


def kernel(hidden, encoder_outputs):
    from concourse.bass_utils import run_bass_kernel_spmd

    nc = get_nc()
    in_maps = make_in_maps(hidden, encoder_outputs)
    res = run_bass_kernel_spmd(nc, in_maps, list(range(NCORES))).results
    return assemble(res)

